# revision 1
# baseline (speedup 1.0000x reference)
"""Distributed Bass kernel for nn_Attn_45372034515281 on 8 TRN2 NeuronCores.

Math (per batch b, head h):
  qkv = x @ w_qkv ; q,k,v head-split
  q = rmsnorm(q)*scaler ; k = rmsnorm(k)*scaler ; rope(q,k)
  S = (q @ k^T) * sqrt(dh)           [n, m]
  P = softmax(S, axis=m)
  colsum[m] = sum_n P[n, m]
  attn[m, :] = v[m, :] * colsum[m]
  out = attn(head-merge) @ w_out + b_out

Sharding: core c -> batch b=c//4, head group g=c%4 (heads 4g..4g+3).
Each core computes scores+softmax colsums for its 4 heads over all n,
then (after a tiny 4-rank AllToAll of colsums) the output rows
[512g : 512g+512] of its batch with the full w_out.

Engine mapping: PE f32r matmuls for qkv/scores (precision: logits have std
~68, bf16 flips argmax -> fails 2e-2 gate; f32r measured 1.6e-4), bf16 for
E-stream/v/out-proj. ACT: exp with fused row-sum. DVE: row-max, rms, rope.
"""
import os
import numpy as np
import ml_dtypes

import concourse.bass as bass
import concourse.bacc as bacc
import concourse.mybir as mybir
import concourse.tile as tile
from concourse.bass_utils import run_bass_kernel_spmd

F32 = mybir.dt.float32
F32R = mybir.dt.float32r
BF16 = mybir.dt.bfloat16
AX = mybir.AxisListType
OP = mybir.AluOpType
ACT = mybir.ActivationFunctionType

B, N, D = 2, 2048, 1024
H, DH = 16, 64
EPS = 1e-6
ROPE_BASE = 10000.0
N_CORES = 8
NT = N // 128          # 16 n-tiles
KT = D // 128          # 8 k-tiles
HL = 4                 # heads per core
ROWS = 512             # output rows per core

TRACE = False          # set by test.py for profiling runs
_CACHE = {}


def _build():
    nc = bacc.Bacc("TRN2", target_bir_lowering=False)

    xT = nc.declare_dram_parameter("xT", [D, N], F32, isOutput=False)
    w_qk = nc.declare_dram_parameter("w_qk", [D, 512], F32, isOutput=False)
    xTv = nc.declare_dram_parameter("xTv", [D, ROWS], BF16, isOutput=False)
    w_v = nc.declare_dram_parameter("w_v", [D, D], BF16, isOutput=False)
    w_out = nc.declare_dram_parameter("w_out", [D, D], BF16, isOutput=False)
    b_outT = nc.declare_dram_parameter("b_outT", [D, 1], F32, isOutput=False)
    ident = nc.declare_dram_parameter("ident", [128, 128], F32, isOutput=False)
    onehot = nc.declare_dram_parameter("onehot", [4, 16], BF16, isOutput=False)
    # rope tables with scaler folded: [N, HL*32] each
    tabs_in = [nc.declare_dram_parameter(f"tab{i}", [N, 128], F32, isOutput=False)
               for i in range(4)]
    yT = nc.declare_dram_parameter("yT", [D, ROWS], F32, isOutput=True)

    with tile.TileContext(nc) as tc:
        with tc.tile_pool(name="const", bufs=1) as cp, \
             tc.tile_pool(name="dram", bufs=1, space="DRAM") as dp:

            # ---- persistent sbuf ----
            id_sb = cp.tile([128, 128], F32, tag="ident")
            nc.sync.dma_start(id_sb[:], ident[:, :])
            # qkT: transposed q,k feature-major [dh-part, n], f32r
            # [0]=q h0,h1  [1]=q h2,h3  [2]=k h0,h1  [3]=k h2,h3
            qkT_all = cp.tile([128, 4 * N], F32R, tag="qkTall")
            qkT = [qkT_all[:, j * N:(j + 1) * N] for j in range(4)]
            # colsum staging: [4 local heads, N] collected in sbuf, scattered to
            # the global head axis via a one-hot matmul (one-hot is per-core
            # input data, so the program stays SPMD-identical), then a 4-rank
            # ReduceScatter-add hands each core all 16 heads x its row quarter.
            cs4_sb = cp.tile([4, N], BF16, tag="cs4")
            oh_sb = cp.tile([4, 16], BF16, tag="oh")
            nc.sync.dma_start(oh_sb[:], onehot[:, :])
            rs_in = dp.tile([4, 16, ROWS], BF16)
            rs_out = dp.tile([16, ROWS], BF16)

            # ================= stage A: qkv proj + rms + rope + transpose ====
            with tc.tile_pool(name="stA", bufs=3) as sa, \
                 tc.tile_pool(name="ldst", bufs=1) as lp, \
                 tc.tile_pool(name="psA", bufs=3, space="PSUM") as psA, \
                 tc.tile_pool(name="psT", bufs=4, space="PSUM") as psT:

                # rope tables: dram [N=16*128, 128] -> sbuf [128, 16, 128]
                tabs = []
                for i in range(4):
                    t_sb = lp.tile([128, NT * 128], F32, tag=f"tab{i}", name=f"tab{i}")
                    tsrc = bass.AP(tabs_in[i][:, :].tensor, 0,
                                   [[128, 128], [128 * 128, NT], [1, 128]])
                    nc.sync.dma_start(t_sb[:].rearrange("p (t d) -> p t d", t=NT), tsrc)
                    tabs.append(t_sb)
                # load + round inputs to f32r (staging slots shared via tag)
                xT_r, wqk_r = [], []
                for k in range(KT):
                    st = lp.tile([128, N], F32, tag="xst", bufs=2, name=f"xst{k}")
                    nc.sync.dma_start(st[:], xT[128 * k:128 * (k + 1), :])
                    xr = lp.tile([128, N], F32R, tag=f"xr{k}", name=f"xr{k}")
                    nc.vector.tensor_copy(xr[:], st[:])
                    xT_r.append(xr)
                    st2 = lp.tile([128, 512], F32, tag="wst", bufs=2, name=f"wst{k}")
                    nc.sync.dma_start(st2[:], w_qk[128 * k:128 * (k + 1), :])
                    wr = lp.tile([128, 512], F32R, tag=f"wr{k}", name=f"wr{k}")
                    nc.vector.tensor_copy(wr[:], st2[:])
                    wqk_r.append(wr)

                for nt in range(NT):
                    ps = psA.tile([128, 512], F32, tag="ps")
                    for k in range(KT):
                        nc.tensor.matmul(ps[:], xT_r[k][:, 128 * nt:128 * (nt + 1)],
                                         wqk_r[k][:], start=(k == 0), stop=(k == KT - 1))
                    # rms stats: ssq per 64-block (square from the sbuf copy;
                    # TT cannot read two PSUM operands)
                    qkf = sa.tile([128, 512], F32, tag="qkf")
                    nc.scalar.copy(qkf[:], ps[:])
                    sq = sa.tile([128, 512], F32, tag="sq")
                    nc.scalar.square(sq[:], ps[:])
                    st8 = sa.tile([128, 8], F32, tag="st8")
                    nc.vector.tensor_reduce(st8[:], sq[:].rearrange("p (g e) -> p g e", g=8),
                                            AX.X, OP.add)
                    # t = ssq/64 + eps  (q blocks fold the *8 score scale: /4096, eps/64)
                    taf = sa.tile([128, 8], F32, tag="taf")
                    nc.vector.tensor_scalar(taf[:, 0:4], st8[:, 0:4],
                                            1.0 / 4096.0, EPS / 64.0, OP.mult, OP.add)
                    nc.vector.tensor_scalar(taf[:, 4:8], st8[:, 4:8],
                                            1.0 / 64.0, EPS, OP.mult, OP.add)
                    # rsqrt via single ACT op (accuracy HW-checked vs rel-err gate)
                    rms = sa.tile([128, 8], F32, tag="rms")
                    nc.scalar.activation(rms[:], taf[:], ACT.Abs_reciprocal_sqrt)

                    # apply rms (free-broadcast over 64)
                    rms_b = bass.AP(rms[:].tensor, rms[:].offset,
                                    [rms[:].ap[0], [1, 8], [0, 64]])
                    nc.vector.tensor_tensor(qkf[:].rearrange("p (g e) -> p g e", g=8),
                                            qkf[:].rearrange("p (g e) -> p g e", g=8),
                                            rms_b, OP.mult)
                    # rope: blocks (2 qk x 4 h); tables broadcast over qk dim
                    def half(off):
                        a = qkf[:]
                        return bass.AP(a.tensor, a.offset + off,
                                       [a.ap[0], [256, 2], [64, 4], [1, 32]])
                    def tab(i):
                        a = tabs[i][:]
                        return bass.AP(a.tensor, a.offset + 128 * nt,
                                       [a.ap[0], [0, 2], [32, 4], [1, 32]])
                    t1, t2 = half(0), half(32)
                    u1 = sa.tile([128, 256], F32, tag="u1")
                    u2 = sa.tile([128, 256], F32, tag="u2")
                    u3 = sa.tile([128, 256], F32, tag="u3")
                    u4 = sa.tile([128, 256], F32, tag="u4")
                    v4 = lambda t: t[:].rearrange("p (a b c) -> p a b c", a=2, b=4)
                    nc.vector.tensor_tensor(v4(u1), t1, tab(0), OP.mult)
                    nc.vector.tensor_tensor(v4(u2), t2, tab(1), OP.mult)
                    nc.gpsimd.tensor_tensor(v4(u3), t1, tab(2), OP.mult)
                    nc.gpsimd.tensor_tensor(v4(u4), t2, tab(3), OP.mult)
                    rot = sa.tile([128, 512], F32, tag="rot")
                    ro = rot[:]
                    o1 = bass.AP(ro.tensor, ro.offset, [ro.ap[0], [256, 2], [64, 4], [1, 32]])
                    o2 = bass.AP(ro.tensor, ro.offset + 32, [ro.ap[0], [256, 2], [64, 4], [1, 32]])
                    nc.gpsimd.tensor_tensor(o1, v4(u1), v4(u2), OP.subtract)
                    nc.gpsimd.tensor_tensor(o2, v4(u3), v4(u4), OP.add)
                    # transpose 4x [128,128] into one psum tile, then copy per
                    # section (a single strided 4-way copy was observed to race)
                    pt = psT.tile([128, 512], F32, tag="pt")
                    for j in range(4):
                        nc.tensor.transpose(pt[:, 128 * j:128 * (j + 1)],
                                            rot[:, 128 * j:128 * (j + 1)], id_sb[:])
                    for j in range(4):
                        nc.scalar.copy(qkT_all[:, j * N + 128 * nt:j * N + 128 * (nt + 1)],
                                       pt[:, 128 * j:128 * (j + 1)])

            # ===== v-proj (hoisted: independent of colsums; fills PE slack) ==
            wc_cm = tc.tile_pool(name="wC", bufs=1)
            wc = wc_cm.__enter__()
            xv_sb = [wc.tile([128, ROWS], BF16, tag=f"xv{k}", name=f"xv{k}") for k in range(KT)]
            wv_sb = [wc.tile([128, D], BF16, tag=f"wv{k}", name=f"wv{k}") for k in range(KT)]
            wo_sb = [wc.tile([128, D], BF16, tag=f"wo{k}", name=f"wo{k}") for k in range(KT)]
            vt_sb = [wc.tile([128, ROWS], F32, tag=f"vt{t}", name=f"vt{t}") for t in range(KT)]
            at_sb = [wc.tile([128, ROWS], BF16, tag=f"at{t}", name=f"at{t}") for t in range(KT)]
            bb = wc.tile([128, 8], F32, tag="bb")
            with tc.tile_pool(name="psV", bufs=2, space="PSUM") as psV:
                for k in range(KT):
                    nc.sync.dma_start(xv_sb[k][:], xTv[128 * k:128 * (k + 1), :])
                    nc.sync.dma_start(wv_sb[k][:], w_v[128 * k:128 * (k + 1), :])
                    nc.sync.dma_start(wo_sb[k][:], w_out[128 * k:128 * (k + 1), :])
                nc.sync.dma_start(bb[:], bass.AP(b_outT[:, :].tensor, 0, [[1, 128], [128, 8]]))
                for t in range(KT):
                    pv = psV.tile([128, ROWS], F32, tag="pv")
                    for k in range(KT):
                        nc.tensor.matmul(pv[:], wv_sb[k][:, 128 * t:128 * (t + 1)],
                                         xv_sb[k][:], start=(k == 0), stop=(k == KT - 1))
                    nc.scalar.copy(vt_sb[t][:], pv[:])

            # ================= stage B: scores + softmax colsum ==============
            with tc.tile_pool(name="stB", bufs=4) as sb_, \
                 tc.tile_pool(name="csb", bufs=1) as csp, \
                 tc.tile_pool(name="psB", bufs=2, space="PSUM") as psB, \
                 tc.tile_pool(name="psC", bufs=1, space="PSUM") as psC:
                for h in range(HL):
                    qt = qkT[h // 2]
                    kt = qkT[2 + h // 2]
                    rp = 64 * (h % 2)
                    pc = psC.tile([1, N], F32, tag="pc")
                    for nt in range(NT):
                        ps1 = psB.tile([128, 1024], F32, tag="psb", name="ps1")
                        ps2 = psB.tile([128, 1024], F32, tag="psb", name="ps2")
                        for mc in range(2):
                            nc.tensor.matmul(ps1[:, 512 * mc:512 * (mc + 1)],
                                             qt[rp:rp + 64, 128 * nt:128 * (nt + 1)],
                                             kt[rp:rp + 64, 512 * mc:512 * (mc + 1)],
                                             start=True, stop=True)
                            nc.tensor.matmul(ps2[:, 512 * mc:512 * (mc + 1)],
                                             qt[rp:rp + 64, 128 * nt:128 * (nt + 1)],
                                             kt[rp:rp + 64, 1024 + 512 * mc:1024 + 512 * (mc + 1)],
                                             start=True, stop=True)
                        # per-half softmax (halves over m): each half uses its own
                        # row-max so exp(h1) never waits on h2; the shift is folded
                        # into the colsum weights rb12 = e^(max_i - M) / Z.
                        nm12 = sb_.tile([128, 2], F32, tag="nm12")
                        nc.vector.tensor_reduce(nm12[:, 0:1], ps1[:], AX.X, OP.max, negate=True)
                        nc.vector.tensor_reduce(nm12[:, 1:2], ps2[:], AX.X, OP.max, negate=True)
                        et = sb_.tile([128, N], BF16, tag="et")
                        z12 = sb_.tile([128, 2], F32, tag="z12")
                        nc.scalar.activation(et[:, 0:1024], ps1[:], ACT.Exp,
                                             bias=nm12[:, 0:1], scale=1.0,
                                             accum_out=z12[:, 0:1])
                        nc.scalar.activation(et[:, 1024:2048], ps2[:], ACT.Exp,
                                             bias=nm12[:, 1:2], scale=1.0,
                                             accum_out=z12[:, 1:2])
                        nmM = sb_.tile([128, 1], F32, tag="nmM")
                        nc.vector.tensor_reduce(nmM[:], nm12[:], AX.X, OP.min)
                        t12 = sb_.tile([128, 2], F32, tag="t12")
                        nmM_b = bass.AP(nmM[:].tensor, nmM[:].offset, [nmM[:].ap[0], [0, 2]])
                        nc.vector.tensor_tensor(t12[:], nmM_b, nm12[:], OP.subtract)
                        ed12 = sb_.tile([128, 2], F32, tag="ed12")
                        nc.scalar.activation(ed12[:], t12[:], ACT.Exp)
                        zw = sb_.tile([128, 2], F32, tag="zw")
                        nc.vector.tensor_tensor(zw[:], z12[:], ed12[:], OP.mult)
                        zf = sb_.tile([128, 1], F32, tag="zf")
                        nc.vector.tensor_reduce(zf[:], zw[:], AX.X, OP.add)
                        rf = sb_.tile([128, 1], F32, tag="rf")
                        nc.vector.reciprocal(rf[:], zf[:])
                        rb12 = sb_.tile([128, 2], BF16, tag="rb12")
                        rf_b = bass.AP(rf[:].tensor, rf[:].offset, [rf[:].ap[0], [0, 2]])
                        nc.vector.tensor_tensor(rb12[:], ed12[:], rf_b, OP.mult)
                        for ccb in range(4):
                            nc.tensor.matmul(pc[0:1, 512 * ccb:512 * (ccb + 1)],
                                             rb12[:, ccb // 2:ccb // 2 + 1],
                                             et[:, 512 * ccb:512 * (ccb + 1)],
                                             start=(nt == 0), stop=(nt == NT - 1))
                    cs1 = csp.tile([1, N], BF16, tag="cs1", name=f"cs1_{h}")
                    nc.scalar.copy(cs1[:], pc[:])
                    nc.sync.dma_start(cs4_sb[h:h + 1, :], cs1[:])

            # ====== colsum scatter (one-hot matmul) + ReduceScatter ==========
            with tc.tile_pool(name="sctr", bufs=1) as scp, \
                 tc.tile_pool(name="psS", bufs=1, space="PSUM") as psS:
                pss = psS.tile([16, N], F32, tag="pss")
                for ccb in range(4):
                    nc.tensor.matmul(pss[:, 512 * ccb:512 * (ccb + 1)], oh_sb[:],
                                     cs4_sb[:, 512 * ccb:512 * (ccb + 1)],
                                     start=True, stop=True)
                sc_sb = scp.tile([16, N], BF16, tag="scs")
                nc.scalar.copy(sc_sb[:], pss[:])
                # rs_in[d, h', r] = sc_sb[h', 512 d + r]
                dst = bass.AP(rs_in[:].tensor, rs_in[:].offset,
                              [[ROWS, 16], [16 * ROWS, 4], [1, ROWS]])
                nc.sync.dma_start(dst, sc_sb[:].rearrange("p (d r) -> p d r", d=4))
            nc.gpsimd.collective_compute(
                "ReduceScatter", OP.add,
                ins=[rs_in[:].rearrange("a b r -> (a b) r").opt()],
                outs=[rs_out[:].opt()],
                replica_groups=[[0, 1, 2, 3], [4, 5, 6, 7]],
            )
            with tc.tile_pool(name="stC", bufs=2) as sc, \
                 tc.tile_pool(name="psO", bufs=1, space="PSUM") as psO:
                for t in range(KT):
                    ce = sc.tile([128, ROWS], BF16, tag="ce")
                    for hh in range(2):
                        src = bass.AP(rs_out[:].tensor,
                                      rs_out[:].offset + (2 * t + hh) * ROWS,
                                      [[0, 64], [1, ROWS]])
                        nc.sync.dma_start(ce[64 * hh:64 * (hh + 1), :], src)
                    nc.vector.tensor_tensor(at_sb[t][:], vt_sb[t][:], ce[:], OP.mult)
                # two rounds of 4 output tiles: k-major accumulation starts as
                # soon as each attnT tile is ready
                for rnd in range(2):
                    pos = [psO.tile([128, ROWS], F32, tag=f"po{j}", name=f"po{rnd}_{j}")
                           for j in range(4)]
                    for k in range(KT):
                        for j in range(4):
                            jj = 4 * rnd + j
                            nc.tensor.matmul(pos[j][:],
                                             wo_sb[k][:, 128 * jj:128 * (jj + 1)],
                                             at_sb[k][:], start=(k == 0),
                                             stop=(k == KT - 1))
                    for j in range(4):
                        jj = 4 * rnd + j
                        yo = sc.tile([128, ROWS], F32, tag="yo", name=f"yo{rnd}_{j}")
                        nc.scalar.activation(yo[:], pos[j][:], ACT.Identity,
                                             bias=bb[:, jj:jj + 1], scale=1.0)
                        nc.sync.dma_start(yT[128 * jj:128 * (jj + 1), :], yo[:])
            wc_cm.__exit__(None, None, None)

    nc.finalize()
    return nc


def _prep_inputs(x, w_qkv, w_out, b_out, scale_param):
    x = np.asarray(x, np.float32)
    w_qkv = np.asarray(w_qkv, np.float32)
    w_out = np.asarray(w_out, np.float32)
    b_out = np.asarray(b_out, np.float32)
    scale_param = np.asarray(scale_param, np.float32)

    scaler = scale_param * (D ** 0.5)                      # [H, DH]
    inv_freq = 1.0 / (ROPE_BASE ** (np.arange(0, DH, 2, dtype=np.float32) / DH))
    ang = np.arange(N, dtype=np.float32)[:, None] * inv_freq[None, :]   # [N, 32]
    cosv, sinv = np.cos(ang), np.sin(ang)

    w_v_bf = w_qkv[:, 2 * D:3 * D].astype(ml_dtypes.bfloat16)
    w_out_bf = w_out.astype(ml_dtypes.bfloat16)
    ident = np.eye(128, dtype=np.float32)
    b_outT = b_out.reshape(D, 1).astype(np.float32)

    in_maps = []
    for c in range(N_CORES):
        b, g = c // 4, c % 4
        xb = np.ascontiguousarray(x[b].T)                  # [D, N]
        wq = w_qkv[:, 256 * g:256 * (g + 1)]
        wk = w_qkv[:, D + 256 * g:D + 256 * (g + 1)]
        w_qk = np.ascontiguousarray(np.concatenate([wq, wk], axis=1))
        xTv = np.ascontiguousarray(xb[:, ROWS * g:ROWS * (g + 1)]).astype(ml_dtypes.bfloat16)
        tabs = []
        for kind in range(4):
            t = np.empty((N, 128), np.float32)
            for hl in range(HL):
                hgl = 4 * g + hl
                s1 = scaler[hgl, 0:32][None, :]
                s2 = scaler[hgl, 32:64][None, :]
                col = {0: cosv * s1, 1: sinv * s2, 2: sinv * s1, 3: cosv * s2}[kind]
                t[:, 32 * hl:32 * (hl + 1)] = col
            tabs.append(t)
        oh = np.zeros((4, 16), ml_dtypes.bfloat16)
        for lh in range(HL):
            oh[lh, 4 * g + lh] = 1.0
        in_maps.append({
            "xT": xb, "w_qk": w_qk, "xTv": xTv, "w_v": w_v_bf,
            "w_out": w_out_bf, "b_outT": b_outT, "ident": ident, "onehot": oh,
            "tab0": tabs[0], "tab1": tabs[1], "tab2": tabs[2], "tab3": tabs[3],
        })
    return in_maps


def kernel(x, w_qkv, w_out, b_out, scale_param):
    if "nc" not in _CACHE:
        _CACHE["nc"] = _build()
    nc = _CACHE["nc"]
    in_maps = _prep_inputs(x, w_qkv, w_out, b_out, scale_param)
    res = run_bass_kernel_spmd(nc, in_maps, core_ids=list(range(N_CORES)),
                               trace=TRACE)
    _CACHE["last_result"] = res
    out = np.empty((B, N, D), np.float32)
    for c in range(N_CORES):
        b, g = c // 4, c % 4
        out[b, ROWS * g:ROWS * (g + 1), :] = res.results[c]["yT"].T
    return out

